# revision 17
# baseline (speedup 1.0000x reference)
"""Trainium2 Bass kernel for nn_BBoxGenerator (segment_reduce).

mask_fg (256, 1, 512, 512) f32 -> boxes (256, 4) f32 [x0, y0, x1, y1].

Pure data parallel: each of the 8 cores handles 32 images independently.

Per-core pipeline (image = SBUF tile (128, 4*512) f32, partition p holds
rows 4p..4p+3):
  - Loads are plain HWDGE f32 DMAs on the SP ring (nc.sync): no SWDGE, so
    descriptor generation never contends with gpsimd work or DVE port
    locks. HBM read stream (32 MB) is the roofline (~368 GB/s).
  - Threshold split across two engines running concurrently:
      DVE (odd images):  mask = (m > 0.5) in {0,1} bf16, fused accum row sums
      ACT (even images): mask = Relu(m - 0.5) in {0} u (0,0.5] bf16, fused
        accum row sums. Both encodings make "any" == (sum > 0), so every
        downstream threshold is a single unified  > 0  compare.
  - PE: col sums via 4 matmuls/image with a one-hot (128,32) stationary
    routing image i to PSUM partition row i (DRAM image order -> no
    un-permute at the end). Two accumulation groups (images 0..15 -> psumA,
    16..31 -> psumB) so group A's column binarize runs mid-stream.
  - ACT binarizes PSUM col counts with Sign (counts are >= 0) into colbits;
    group A mid-stream, group B in the tail.
  - Row side: masked min/max of row index built from the rc accum sums;
    bulk pass (images 0..27) on gpsimd (stt) + DVE (reduce) mid-stream,
    last 4 images in the tail. One TensorE transpose for the
    cross-partition row reduction.
  - x side: fused tensor_tensor_reduce (colbits * (w-512) -> min,
    colbits * (w+1) -> max) straight into the raw box tile.
  - Box math on (32,4) raw integer coords; expand + empty default; single
    512B output DMA in image order.
"""

import numpy as np

from concourse import bacc, mybir
from concourse.tile import TileContext
from concourse.bass_utils import run_bass_kernel_spmd

F32 = mybir.dt.float32
BF16 = mybir.dt.bfloat16
I32 = mybir.dt.int32
OP = mybir.AluOpType
AX = mybir.AxisListType
AF = mybir.ActivationFunctionType

N_CORES = 8
B = 256
BP = B // N_CORES  # 32 images per core
H = W = 512
IMG_FREE = 4 * W  # 2048 free elems per image (4 rows per partition)

MIN_BOX = 0.05
GRP = 16  # images per PSUM accumulation group
E1 = 28  # images covered by the early (mid-stream) row pass
TAIL_SPLIT = 2  # how many trailing images get row-sliced DMAs


def build_nc():
    nc = bacc.Bacc("TRN2", target_bir_lowering=False, debug=False, num_devices=N_CORES)
    x = nc.declare_dram_parameter("mask_fg", [BP, 1, H, W], F32, isOutput=False)
    out = nc.declare_dram_parameter("out", [BP, 4], F32, isOutput=True)

    # (128, BP, 4, 512): partition p holds rows 4p..4p+3 of each image
    xv = x.ap().rearrange("b one (p a) w -> p (b one) a w", p=128)

    with TileContext(nc) as tc:
        with (
            tc.tile_pool(name="consts", bufs=1) as consts,
            tc.tile_pool(name="imgs", bufs=16) as imgs,
            tc.tile_pool(name="masks", bufs=4) as masks,
            tc.tile_pool(name="small", bufs=1) as small,
            tc.tile_pool(name="pA", bufs=1, space="PSUM") as pA_pool,
            tc.tile_pool(name="pB", bufs=1, space="PSUM") as pB_pool,
            tc.tile_pool(name="ptr", bufs=1, space="PSUM") as ptr_pool,
        ):
            # ---- constants (gpsimd only; HWDGE loads don't care) ----
            neg_half = consts.tile([128, 1], F32)
            nc.gpsimd.memset(neg_half[:], -0.5)

            # one-hot stationary: image k -> psum row k % GRP of its group's
            # tile (PSUM reads must be 32-partition aligned, so both groups
            # land in rows 0..15 of their own bank)
            oh_ones = consts.tile([128, BP * 32], BF16)
            nc.gpsimd.memset(oh_ones[:], 1.0)
            oh = consts.tile([128, BP * 32], BF16)
            nc.gpsimd.affine_select(
                oh[:].rearrange("p (g k j) -> p g k j", g=BP // GRP, j=32),
                oh_ones[:].rearrange("p (g k j) -> p g k j", g=BP // GRP, j=32),
                [[0, BP // GRP], [-1, GRP], [1, 32]], OP.is_equal, 0.0,
                base=0, channel_multiplier=0,
            )

            # hm[p, 4k+r] = 4p + r - 512 ; hp = 4p + r + 1  (row index consts)
            hm_i = consts.tile([128, 4 * BP], I32)
            nc.gpsimd.iota(hm_i[:], [[0, BP], [1, 4]], base=-512, channel_multiplier=4)
            hm = consts.tile([128, 4 * BP], F32)
            nc.gpsimd.tensor_copy(hm[:], hm_i[:])
            hp_i = consts.tile([128, 4 * BP], I32)
            nc.gpsimd.iota(hp_i[:], [[0, BP], [1, 4]], base=1, channel_multiplier=4)
            hp = consts.tile([128, 4 * BP], F32)
            nc.gpsimd.tensor_copy(hp[:], hp_i[:])

            # wm[i, w] = w - 512 ; wp = w + 1  (col index consts)
            wm_i = consts.tile([48, W], I32)
            nc.gpsimd.iota(wm_i[:], [[1, W]], base=-512, channel_multiplier=0)
            wm = consts.tile([48, W], F32)
            nc.gpsimd.tensor_copy(wm[:], wm_i[:])
            wp_i = consts.tile([48, W], I32)
            nc.gpsimd.iota(wp_i[:], [[1, W]], base=1, channel_multiplier=0)
            wp = consts.tile([48, W], F32)
            nc.gpsimd.tensor_copy(wp[:], wp_i[:])

            ones128 = consts.tile([128, 128], F32)
            nc.gpsimd.memset(ones128[:], 1.0)
            ident = consts.tile([128, 128], F32)
            nc.gpsimd.affine_select(
                ident[:], ones128[:], [[-1, 128]], OP.is_equal, 0.0,
                base=0, channel_multiplier=1,
            )

            # boxes = raw/512 + offs2 ; default box
            offs2 = consts.tile([48, 4], F32)
            nc.gpsimd.memset(offs2[:, 0:2], 1.0)
            nc.gpsimd.memset(offs2[:, 2:4], -1.0 / 512)
            dflt = consts.tile([48, 4], F32)
            nc.gpsimd.memset(dflt[:, 0:2], 0.25)
            nc.gpsimd.memset(dflt[:, 2:4], 0.75)

            # ---- working tiles ----
            # partition row for image k in the (48,*) finishing tiles:
            # k + 16*(k >= 16), i.e. group A -> rows 0..15, B -> rows 32..47
            # (all engine accesses stay 32-partition aligned).
            rc = small.tile([128, 4 * BP], F32)  # row sums, col 4i+r
            rtmp_min = small.tile([128, 4 * BP], F32)
            rtmp_max = small.tile([128, 4 * BP], F32)
            rvals = small.tile([128, 128], F32)  # col prow(k): min, 64+prow(k): max
            nc.gpsimd.memset(rvals[:], 0.0)
            colbits = small.tile([48, W], F32)  # row prow(i): col-any bits
            nc.gpsimd.memset(colbits[:], 0.0)
            psumA = pA_pool.tile([32, W], F32)
            psumB = pB_pool.tile([32, W], F32)

            # ---- main loop ----
            for i in range(BP):
                img = imgs.tile([128, IMG_FREE], F32)
                if i >= BP - TAIL_SPLIT:
                    # row-sliced loads so tail thresholds start early
                    for r in range(4):
                        nc.sync.dma_start(
                            out=img[:, r * W:(r + 1) * W],
                            in_=xv[:, i:i + 1, r:r + 1],
                        )
                else:
                    nc.sync.dma_start(
                        out=img[:].rearrange("p (a w) -> p a w", a=4),
                        in_=xv[:, i:i + 1],
                    )
                m01 = masks.tile([128, IMG_FREE], BF16,
                                 tag="m01a" if i % 2 == 0 else "m01d")
                for r in range(4):
                    sl = slice(r * W, (r + 1) * W)
                    acc = rc[:, 4 * i + r:4 * i + r + 1]
                    if i % 2 == 0:
                        nc.scalar.activation(
                            m01[:, sl], img[:, sl], AF.Relu,
                            bias=neg_half[:], accum_out=acc,
                        )
                    else:
                        nc.vector.tensor_scalar(
                            m01[:, sl], img[:, sl], 0.5, None,
                            OP.is_gt, OP.add, accum_out=acc,
                        )
                ps = psumA if i < GRP else psumB
                for r in range(4):
                    sl = slice(r * W, (r + 1) * W)
                    nc.tensor.matmul(
                        ps[:, :], oh[:, 32 * i:32 * (i + 1)], m01[:, sl],
                        start=(i % GRP == 0 and r == 0),
                        stop=(i % GRP == GRP - 1 and r == 3),
                    )
                if i == GRP + 2:
                    # group A col binarize mid-stream (counts >= 0: Sign -> {0,1})
                    nc.scalar.activation(colbits[0:GRP, :], psumA[0:GRP, :], AF.Sign)
                if i == E1:
                    # early row pass (images 0..E1-1): masked row-index values
                    # (codegen rejects stt on Pool, so these ride DVE's slack)
                    nc.vector.scalar_tensor_tensor(
                        rtmp_min[:, 0:4 * E1], rc[:, 0:4 * E1], 0.0,
                        hm[:, 0:4 * E1], OP.is_gt, OP.mult)
                    nc.vector.scalar_tensor_tensor(
                        rtmp_max[:, 0:4 * E1], rc[:, 0:4 * E1], 0.0,
                        hp[:, 0:4 * E1], OP.is_gt, OP.mult)
                if i == E1 + 1:
                    # the groupwise X-reduces must run on DVE; dst columns
                    # split at image 16 (group B lands at col 32+)
                    for dst0, rt, op in ((0, rtmp_min, OP.min), (64, rtmp_max, OP.max)):
                        nc.vector.tensor_reduce(
                            rvals[:, dst0:dst0 + GRP],
                            rt[:, 0:4 * GRP].rearrange("p (i r) -> p i r", r=4),
                            op=op, axis=AX.X)
                        nc.vector.tensor_reduce(
                            rvals[:, dst0 + 32:dst0 + 32 + (E1 - GRP)],
                            rt[:, 4 * GRP:4 * E1].rearrange("p (i r) -> p i r", r=4),
                            op=op, axis=AX.X)

            # ---- finishing (tail) ----
            # row pass 2: images E1..31 land at rvals cols 16+E1 .. 16+BP
            nc.vector.scalar_tensor_tensor(
                rtmp_min[:, 4 * E1:], rc[:, 4 * E1:], 0.0,
                hm[:, 4 * E1:], OP.is_gt, OP.mult)
            nc.vector.tensor_reduce(
                rvals[:, 16 + E1:16 + BP],
                rtmp_min[:, 4 * E1:].rearrange("p (i r) -> p i r", r=4),
                op=OP.min, axis=AX.X)
            nc.vector.scalar_tensor_tensor(
                rtmp_max[:, 4 * E1:], rc[:, 4 * E1:], 0.0,
                hp[:, 4 * E1:], OP.is_gt, OP.mult)
            nc.vector.tensor_reduce(
                rvals[:, 80 + E1:80 + BP],
                rtmp_max[:, 4 * E1:].rearrange("p (i r) -> p i r", r=4),
                op=OP.max, axis=AX.X)

            rT = ptr_pool.tile([128, 128], F32)
            nc.tensor.transpose(rT[:], rvals[:], ident[:])

            # group B col binarize (group B routes to rows 0..15 of its bank,
            # colbits rows 32..47)
            nc.scalar.activation(colbits[32:48, :], psumB[0:GRP, :], AF.Sign)

            raw = small.tile([48, 4], F32)  # [x_min-512, y_min-512, x_max+1, y_max+1]
            ctmp = small.tile([48, W], F32)
            ctmp2 = small.tile([48, W], F32)
            nc.vector.tensor_mul(ctmp[:], colbits[:], wm[:])
            nc.vector.tensor_reduce(raw[:, 0:1], ctmp[:], op=OP.min, axis=AX.X)
            nc.vector.tensor_mul(ctmp2[:], colbits[:], wp[:])
            nc.vector.tensor_reduce(raw[:, 2:3], ctmp2[:], op=OP.max, axis=AX.X)
            nc.vector.tensor_reduce(raw[:, 1:2], rT[0:48, :], op=OP.min, axis=AX.X)
            nc.vector.tensor_reduce(raw[:, 3:4], rT[64:112, :], op=OP.max, axis=AX.X)

            # ---- box math on raw integer coords (rows 16..31 are don't-care) ----
            emp = small.tile([48, 1], F32)
            nc.vector.tensor_scalar(emp[:], raw[:, 3:4], 0.5, None, OP.is_lt)
            boxes = small.tile([48, 4], F32)
            nc.vector.scalar_tensor_tensor(
                boxes[:], raw[:], 1.0 / 512, offs2[:], OP.mult, OP.add)

            # expand too-small boxes; all thresholds exact on integer raws:
            # size < 0.05  <=>  (raw_hi - raw_lo) < 0.05*512 + 513
            d_t = small.tile([48, 2], F32)
            too_t = small.tile([48, 2], I32)
            s_t = small.tile([48, 2], F32)
            lo2_t = small.tile([48, 2], F32)
            hi2_t = small.tile([48, 2], F32)
            nc.vector.tensor_sub(d_t[:], raw[:, 2:4], raw[:, 0:2])
            nc.vector.tensor_scalar(
                too_t[:], d_t[:], MIN_BOX * 512 + 513, None, OP.is_lt)
            nc.vector.tensor_add(s_t[:], raw[:, 2:4], raw[:, 0:2])
            # c = (s+511)/1024 ; lo2 = clamp0(c - s/2) ; hi2 = clamp1(c + s/2)
            nc.vector.tensor_scalar(
                lo2_t[:], s_t[:], 1.0 / 1024, 511.0 / 1024 - MIN_BOX / 2,
                OP.mult, OP.add)
            nc.vector.tensor_scalar(lo2_t[:], lo2_t[:], 0.0, None, OP.max)
            nc.vector.tensor_scalar(
                hi2_t[:], s_t[:], 1.0 / 1024, 511.0 / 1024 + MIN_BOX / 2,
                OP.mult, OP.add)
            nc.vector.tensor_scalar(hi2_t[:], hi2_t[:], 1.0, None, OP.min)
            nc.vector.copy_predicated(boxes[:, 0:2], too_t[:], lo2_t[:])
            nc.vector.copy_predicated(boxes[:, 2:4], too_t[:], hi2_t[:])

            # default box where empty: final = (default - boxes) * emp + boxes
            dmb = small.tile([48, 4], F32)
            nc.vector.tensor_sub(dmb[:], dflt[:], boxes[:])
            final = small.tile([48, 4], F32)
            nc.vector.scalar_tensor_tensor(
                final[:], dmb[:], emp[:], boxes[:], OP.mult, OP.add)

            # rows 0..15 -> images 0..15, rows 32..47 -> images 16..31
            nc.sync.dma_start(out=out.ap()[0:GRP], in_=final[0:GRP, :])
            nc.sync.dma_start(out=out.ap()[GRP:BP], in_=final[32:48, :])

    return nc


_NC = None


def _get_nc():
    global _NC
    if _NC is None:
        nc = build_nc()
        nc.compile()
        _NC = nc
    return _NC


def kernel(mask_fg: np.ndarray) -> np.ndarray:
    mask_fg = np.ascontiguousarray(np.asarray(mask_fg, dtype=np.float32))
    assert mask_fg.shape == (B, 1, H, W), mask_fg.shape
    nc = _get_nc()
    shards = mask_fg.reshape(N_CORES, BP, 1, H, W)
    in_maps = [{"mask_fg": np.ascontiguousarray(shards[i])} for i in range(N_CORES)]
    res = run_bass_kernel_spmd(nc, in_maps, core_ids=list(range(N_CORES)))
    return np.concatenate(
        [res.results[i]["out"] for i in range(N_CORES)], axis=0
    ).astype(np.float32)
